# revision 9
# baseline (speedup 1.0000x reference)
"""Trainium2 Bass kernel for nn_CoordinateGCN (8-layer GCN, tridiagonal adjacency).

Strategy
--------
Pure data parallel over the batch: 64 items -> 8 NeuronCores x 8 items.
On-chip layout is feature-major ("transposed"): activations live in SBUF as
x[d, n] with the 1024-dim feature axis on partitions (8 chunks of 128) and the
600 nodes on the free axis.  Every matmul is then native:
    z[e, n] += W[d_chunk, e_tile].T @ rhs[d_chunk, n]     (PSUM fp32 accumulate)
The tridiagonal adjacency is two shifted elementwise adds (never a matmul).
LayerNorm runs in this layout via ones-matmul partition reductions and
ones-outer-product broadcasts.  gelu(gamma*t+beta) is a single ScalarE op per
e-tile (gamma/beta are per-partition scalars).  All matmul operands are bf16
(1 cycle/row on the PE vs 4 for fp32) with fp32 PSUM accumulation.

The host wrapper transposes/casts inputs, folds bp + pos_tab[positions] into a
per-item additive table, and re-assembles the full fp32 output.
"""

import sys

sys.path.insert(0, "/opt/trn_rl_repo")

import numpy as np
import ml_dtypes

BF16 = ml_dtypes.bfloat16

# Problem shapes (hardcoded per the harness contract).
B = 64
NCORES = 8
ITEMS = B // NCORES
P = 128
D = 1024  # input dim == embed dim
KD = D // P
E = 1024
KE = E // P
N = 600
NP = 604  # padded node columns; data at [2, 602), zeros at cols 1 and 602
COL0 = 2
L = 8
CH = 300
NCH = 2
LN_EPS = 1e-5

_CACHE = {}


def _build_nc():
    from contextlib import ExitStack

    import concourse.bass as bass  # noqa: F401
    import concourse.tile as tile
    from concourse import bacc
    import concourse.mybir as mybir

    dt = mybir.dt
    F = mybir.ActivationFunctionType
    OP = mybir.AluOpType

    nc = bacc.Bacc("TRN2", target_bir_lowering=False, debug=False, num_devices=NCORES)

    featT = nc.dram_tensor(
        "featT", [ITEMS, KD, P, N], dt.bfloat16, kind="ExternalInput"
    ).ap()
    posb = nc.dram_tensor(
        "posb", [ITEMS, KE, P, N], dt.bfloat16, kind="ExternalInput"
    ).ap()
    # wts[0] = Wp (input projection), wts[1..L] = per-layer GCN weights
    wts = nc.dram_tensor(
        "wts", [L + 1, KD, P, E], dt.bfloat16, kind="ExternalInput"
    ).ap()
    blv = nc.dram_tensor("blv", [L, 1, E], dt.bfloat16, kind="ExternalInput").ap()
    gam = nc.dram_tensor("gam", [L, P, KE], dt.float32, kind="ExternalInput").ap()
    bet = nc.dram_tensor("bet", [L, P, KE], dt.float32, kind="ExternalInput").ap()
    wo = nc.dram_tensor("wo", [KD, P, 2], dt.bfloat16, kind="ExternalInput").ap()
    bo = nc.dram_tensor("bo", [2, 1], dt.float32, kind="ExternalInput").ap()
    outT = nc.dram_tensor("outT", [ITEMS, 2, N], dt.float32, kind="ExternalOutput").ap()

    def chv(ap):
        # [.., NCH*CH] -> [.., NCH, CH] view so shapes line up with psum tiles
        return ap.rearrange("p (c n) -> p c n", c=NCH)

    with tile.TileContext(nc) as tc, ExitStack() as ctx:
        const = ctx.enter_context(tc.tile_pool(name="const", bufs=1))
        xpool = ctx.enter_context(tc.tile_pool(name="xres", bufs=1))
        wpool = ctx.enter_context(tc.tile_pool(name="wpool", bufs=2))
        lscal = ctx.enter_context(tc.tile_pool(name="lscal", bufs=2))
        aggp = ctx.enter_context(tc.tile_pool(name="aggp", bufs=2))
        zpool = ctx.enter_context(tc.tile_pool(name="zpool", bufs=2))
        z2pool = ctx.enter_context(tc.tile_pool(name="z2pool", bufs=2))
        bcp = ctx.enter_context(tc.tile_pool(name="bcp", bufs=2))
        smp = ctx.enter_context(tc.tile_pool(name="smp", bufs=1))
        obp = ctx.enter_context(tc.tile_pool(name="obp", bufs=2))
        pz = ctx.enter_context(tc.tile_pool(name="pz", bufs=2, space="PSUM"))
        pst = ctx.enter_context(tc.tile_pool(name="pst", bufs=2, space="PSUM"))

        ones_col = const.tile([P, 1], dt.bfloat16)
        nc.vector.memset(ones_col[:], 1.0)
        ones_row = const.tile([1, P], dt.bfloat16)
        nc.vector.memset(ones_row[:], 1.0)
        ones_n = const.tile([1, CH], dt.bfloat16)
        nc.vector.memset(ones_n[:], 1.0)
        eps_sb = const.tile([1, 1], dt.float32)
        nc.vector.memset(eps_sb[:], LN_EPS)
        bo_sb = const.tile([2, 1], dt.float32)
        nc.sync.dma_start(bo_sb[:], bo)
        wo_sb = const.tile([P, KD, 2], dt.bfloat16)
        nc.sync.dma_start(wo_sb[:], wo.rearrange("k p c -> p k c"))

        # Residual stream, resident for all 8 items: [P, item, d_chunk, node]
        x = xpool.tile([P, ITEMS, KD, NP], dt.bfloat16)
        nc.vector.memset(x[:], 0.0)

        # Per-item deferred "phase B" (broadcast + apply) emitted one item late
        # so the PE always has the next item's matmuls queued while the small
        # LN stat chain of the previous item drains.
        pending = []

        def phase_b(st):
            (l, it, z_sb, mu_sb, rstd_sb, ga_sb, be_sb) = st
            mub_ps = pz.tile([P, NCH, 512], dt.float32, tag="zps")
            rsb_ps = pz.tile([P, NCH, 512], dt.float32, tag="zps")
            for c in range(NCH):
                nc.tensor.matmul(
                    mub_ps[:, c, 0:CH],
                    lhsT=ones_row[0:1, :],
                    rhs=mu_sb[0:1, c * CH : (c + 1) * CH],
                    start=True,
                    stop=True,
                )
                nc.tensor.matmul(
                    rsb_ps[:, c, 0:CH],
                    lhsT=ones_row[0:1, :],
                    rhs=rstd_sb[0:1, c * CH : (c + 1) * CH],
                    start=True,
                    stop=True,
                )
            mu_b = bcp.tile([P, N], dt.bfloat16, tag="mub")
            nc.vector.tensor_copy(chv(mu_b[:]), mub_ps[:, :, 0:CH])
            rstd_b = bcp.tile([P, N], dt.bfloat16, tag="rsb")
            nc.vector.tensor_copy(chv(rstd_b[:]), rsb_ps[:, :, 0:CH])

            t = z2pool.tile([P, KD, N], dt.bfloat16, tag="z2")
            nc.vector.tensor_sub(
                t[:], z_sb[:], mu_b[:, None, :].to_broadcast((P, KD, N))
            )
            t2 = aggp.tile([P, KD, N], dt.bfloat16, tag="agg")
            nc.vector.tensor_mul(
                t2[:], t[:], rstd_b[:, None, :].to_broadcast((P, KD, N))
            )
            for ke in range(KE):
                nc.scalar.activation(
                    x[:, it, ke, COL0 : COL0 + N],
                    t2[:, ke, :],
                    F.Gelu,
                    bias=be_sb[:, ke : ke + 1],
                    scale=ga_sb[:, ke : ke + 1],
                )

        for l in range(L + 1):
            w_sb = wpool.tile([P, KD, E], dt.bfloat16, tag="w")
            nc.sync.dma_start(w_sb[:], wts[l].rearrange("k p e -> p k e"))
            if l > 0:
                bl_sb = lscal.tile([1, E], dt.bfloat16, tag="bl")
                nc.sync.dma_start(bl_sb[:], blv[l - 1])
                ga_sb = lscal.tile([P, KE], dt.float32, tag="ga")
                nc.sync.dma_start(ga_sb[:], gam[l - 1])
                be_sb = lscal.tile([P, KE], dt.float32, tag="be")
                nc.sync.dma_start(be_sb[:], bet[l - 1])

            for it in range(ITEMS):
                # ---- phase A: agg, main matmul, residual, stats ----
                if l > 0:
                    agg = aggp.tile([P, KD, N], dt.bfloat16, tag="agg")
                    # shifted neighbor sum (1x mode: misaligned) on GpSimd
                    nc.gpsimd.tensor_tensor(
                        agg[:],
                        x[:, it, :, COL0 - 1 : COL0 - 1 + N],
                        x[:, it, :, COL0 + 1 : COL0 + 1 + N],
                        op=OP.add,
                    )
                    # center add (aligned, 2x mode) on VectorE
                    nc.vector.tensor_tensor(
                        agg[:], agg[:], x[:, it, :, COL0 : COL0 + N], op=OP.add
                    )
                    z_sb = zpool.tile([P, KD, N], dt.bfloat16, tag="z")
                else:
                    # input projection reads features from a separate tile —
                    # x[it] is the *output* and cannot alias the matmul rhs
                    agg = zpool.tile([P, KD, N], dt.bfloat16, tag="z")
                    nc.sync.dma_start(agg[:], featT[it].rearrange("k p n -> p k n"))
                    z_sb = None
                    pb_sb = z2pool.tile([P, KE, N], dt.bfloat16, tag="z2")
                    nc.sync.dma_start(pb_sb[:], posb[it].rearrange("k p n -> p k n"))

                for ke in range(KE):
                    zps = pz.tile([P, NCH, 512], dt.float32, tag="zps")
                    for c in range(NCH):
                        for k in range(KD):
                            rhs = agg[:, k, c * CH : (c + 1) * CH]
                            nc.tensor.matmul(
                                zps[:, c, 0:CH],
                                lhsT=w_sb[:, k, ke * P : (ke + 1) * P],
                                rhs=rhs,
                                start=(k == 0),
                                stop=(l == 0 and k == KD - 1),
                            )
                        if l > 0:
                            # += bl[e] via rank-1 matmul (K=1, rhs=ones)
                            nc.tensor.matmul(
                                zps[:, c, 0:CH],
                                lhsT=bl_sb[0:1, ke * P : (ke + 1) * P],
                                rhs=ones_n[0:1, 0:CH],
                                start=False,
                                stop=True,
                            )
                    if l == 0:
                        nc.vector.tensor_tensor(
                            chv(x[:, it, ke, COL0 : COL0 + N]),
                            zps[:, :, 0:CH],
                            chv(pb_sb[:, ke, :]),
                            op=OP.add,
                        )
                    else:
                        nc.vector.tensor_tensor(
                            chv(z_sb[:, ke, :]),
                            zps[:, :, 0:CH],
                            chv(x[:, it, ke, COL0 : COL0 + N]),
                            op=OP.add,
                        )

                if l == 0:
                    continue

                z2 = z2pool.tile([P, KD, N], dt.bfloat16, tag="z2")
                nc.gpsimd.tensor_tensor(z2[:], z_sb[:], z_sb[:], op=OP.mult)

                sps = pst.tile([1, NCH, 512], dt.float32, tag="sps")
                qps = pst.tile([1, NCH, 512], dt.float32, tag="sps")
                for c in range(NCH):
                    for k in range(KD):
                        nc.tensor.matmul(
                            sps[0:1, c, 0:CH],
                            lhsT=ones_col[:, 0:1],
                            rhs=z_sb[:, k, c * CH : (c + 1) * CH],
                            start=(k == 0),
                            stop=(k == KD - 1),
                        )
                    for k in range(KD):
                        nc.tensor.matmul(
                            qps[0:1, c, 0:CH],
                            lhsT=ones_col[:, 0:1],
                            rhs=z2[:, k, c * CH : (c + 1) * CH],
                            start=(k == 0),
                            stop=(k == KD - 1),
                        )

                mu_sb = smp.tile([1, N], dt.bfloat16, tag="mu")
                nc.vector.tensor_scalar_mul(chv(mu_sb[:]), sps[0:1, :, 0:CH], 1.0 / D)
                sqn_sb = smp.tile([1, N], dt.float32, tag="sqn")
                nc.vector.tensor_scalar_mul(chv(sqn_sb[:]), qps[0:1, :, 0:CH], 1.0 / D)
                m2_sb = smp.tile([1, N], dt.float32, tag="m2")
                nc.vector.tensor_mul(m2_sb[:], mu_sb[:], mu_sb[:])
                var_sb = smp.tile([1, N], dt.float32, tag="var")
                nc.vector.tensor_sub(var_sb[:], sqn_sb[:], m2_sb[:])
                # rstd = exp(-0.5 * ln(var + eps)); Rsqrt is API-banned on ACT
                lnv_sb = smp.tile([1, N], dt.float32, tag="lnv")
                nc.scalar.activation(lnv_sb[:], var_sb[:], F.Ln, bias=eps_sb[0:1, 0:1])
                rstd_sb = smp.tile([1, N], dt.bfloat16, tag="rstd")
                nc.scalar.activation(rstd_sb[:], lnv_sb[:], F.Exp, scale=-0.5)

                if pending:
                    phase_b(pending.pop())
                pending.append((l, it, z_sb, mu_sb, rstd_sb, ga_sb, be_sb))
            # flush the lagging item at layer end (keeps layer weights simple)
            while pending:
                phase_b(pending.pop())

        # output head: coords.T = Wo.T @ x  -> [2, 600] per item, + bo
        for it in range(ITEMS):
            cps = pz.tile([P, NCH, 512], dt.float32, tag="zps")
            for c in range(NCH):
                for k in range(KD):
                    nc.tensor.matmul(
                        cps[0:2, c, 0:CH],
                        lhsT=wo_sb[:, k, :],
                        rhs=x[:, it, k, COL0 + c * CH : COL0 + (c + 1) * CH],
                        start=(k == 0),
                        stop=(k == KD - 1),
                    )
            ob = obp.tile([2, N], dt.float32, tag="ob")
            nc.scalar.activation(
                ob[:].rearrange("p (c n) -> p c n", c=NCH),
                cps[0:2, :, 0:CH],
                F.Identity,
                bias=bo_sb[:, 0:1],
            )
            nc.sync.dma_start(outT[it], ob[:])

    nc.compile()
    return nc


def _get_nc():
    if "nc" not in _CACHE:
        _CACHE["nc"] = _build_nc()
    return _CACHE["nc"]


def _prep_inputs(features, positions, Wp, bp, pos_tab, Wl, bl, gamma, beta, Wo, bo):
    """Host-side packing: transpose/cast to the device layouts."""
    features = np.ascontiguousarray(np.asarray(features, np.float32))
    positions = np.asarray(positions)
    Wp = np.asarray(Wp, np.float32)
    bp = np.asarray(bp, np.float32)
    pos_tab = np.asarray(pos_tab, np.float32)
    Wl = np.asarray(Wl, np.float32)
    bl = np.asarray(bl, np.float32)
    gamma = np.asarray(gamma, np.float32)
    beta = np.asarray(beta, np.float32)
    Wo = np.asarray(Wo, np.float32)
    bo = np.asarray(bo, np.float32)

    featT = (
        features.transpose(0, 2, 1).reshape(B, KD, P, N).astype(BF16)
    )  # [B, k, p, n]
    # bp + pos_tab[positions]: [B, n, e] -> transposed/bf16 per item
    pe = pos_tab[positions] + bp[None, None, :]
    posbT = pe.transpose(0, 2, 1).reshape(B, KE, P, N).astype(BF16)

    wts = np.concatenate([Wp[None], Wl], axis=0)  # [L+1, d, e]
    wts = wts.reshape(L + 1, KD, P, E).astype(BF16)
    blv = bl.reshape(L, 1, E).astype(BF16)
    gam = np.ascontiguousarray(gamma.reshape(L, KE, P).transpose(0, 2, 1))  # [L, P, KE]
    bet = np.ascontiguousarray(beta.reshape(L, KE, P).transpose(0, 2, 1))
    woT = Wo.reshape(KD, P, 2).astype(BF16)
    bov = bo.reshape(2, 1)

    in_maps = []
    for c in range(NCORES):
        sl = slice(c * ITEMS, (c + 1) * ITEMS)
        in_maps.append(
            {
                "featT": np.ascontiguousarray(featT[sl]),
                "posb": np.ascontiguousarray(posbT[sl]),
                "wts": wts,
                "blv": blv,
                "gam": gam,
                "bet": bet,
                "wo": woT,
                "bo": bov,
            }
        )
    return in_maps


def run_device(in_maps, trace=False, **kwargs):
    """Compile (cached) and run the SPMD kernel; returns (results, BassKernelResults)."""
    from concourse import bass_utils

    nc = _get_nc()
    res = bass_utils.run_bass_kernel_spmd(
        nc, in_maps, core_ids=list(range(NCORES)), trace=trace, **kwargs
    )
    return res


def kernel(**inputs) -> np.ndarray:
    in_maps = _prep_inputs(
        inputs["features"],
        inputs["positions"],
        inputs["Wp"],
        inputs["bp"],
        inputs["pos_tab"],
        inputs["Wl"],
        inputs["bl"],
        inputs["gamma"],
        inputs["beta"],
        inputs["Wo"],
        inputs["bo"],
    )
    res = run_device(in_maps, trace=False)
    out = np.empty((B, 600, 2), np.float32)
    for c in range(NCORES):
        o = res.results[c]["outT"]  # [ITEMS, 2, N]
        out[c * ITEMS : (c + 1) * ITEMS] = o.transpose(0, 2, 1)
    out[:, 0, :] = [0.0, 0.0]
    out[:, -1, :] = [600.0, 0.0]
    return out


# revision 28
# speedup vs baseline: 14.2254x; 14.2254x over previous
"""Trainium2 Bass kernel for nn_CoordinateGCN (8-layer GCN, tridiagonal adjacency).

Strategy
--------
Pure data parallel over the batch: 64 items -> 8 NeuronCores x 8 items.
On-chip layout is feature-major ("transposed"): activations live in SBUF as
x[d, n] with the 1024-dim feature axis on partitions (8 chunks of 128) and the
600 nodes on the free axis.  Every matmul is then native:
    z[e, n] += W[d_chunk, e_tile].T @ rhs[d_chunk, n]     (PSUM fp32 accumulate)
The tridiagonal adjacency is two shifted elementwise adds (never a matmul).
LayerNorm runs in this layout via ones-matmul partition reductions and
ones-outer-product broadcasts; variance is computed from the centered tensor
(var = mean(t^2), t = z - mu_broadcast) which the LN apply needs anyway.
gelu(gamma*t+beta) is a single ScalarE op per e-tile (gamma/beta are
per-partition scalars).  All matmul operands are bf16 (1 cycle/row on the PE
vs 4 for fp32) with fp32 PSUM accumulation.

The host wrapper transposes/casts inputs, folds bp + pos_tab[positions] into a
per-item additive table, and re-assembles the full fp32 output.
"""

import sys

sys.path.insert(0, "/opt/trn_rl_repo")

import numpy as np
import ml_dtypes

BF16 = ml_dtypes.bfloat16

# Problem shapes (hardcoded per the harness contract).
B = 64
NCORES = 8
ITEMS = B // NCORES
P = 128
D = 1024  # input dim == embed dim
KD = D // P
E = 1024
KE = E // P
N = 600
NP = 604  # padded node columns; data at [2, 602), zeros at cols 1 and 602
COL0 = 2
L = 8
CH = 300
NCH = 2
SCH = (512, 88)  # stats chunking: each chunk stays inside one PSUM bank
LN_EPS = 1e-5

_CACHE = {}


def _build_nc():
    from contextlib import ExitStack

    import concourse.bass as bass  # noqa: F401
    import concourse.tile as tile
    from concourse import bacc
    import concourse.mybir as mybir

    dt = mybir.dt
    F = mybir.ActivationFunctionType
    OP = mybir.AluOpType

    nc = bacc.Bacc("TRN2", target_bir_lowering=False, debug=False, num_devices=NCORES)

    featT = nc.dram_tensor(
        "featT", [ITEMS, KD, P, N], dt.bfloat16, kind="ExternalInput"
    ).ap()
    posb = nc.dram_tensor(
        "posb", [ITEMS, KE, P, N], dt.bfloat16, kind="ExternalInput"
    ).ap()
    # wts[0] = Wp (input projection), wts[1..L] = per-layer GCN weights
    wts = nc.dram_tensor(
        "wts", [L + 1, KD, P, E], dt.bfloat16, kind="ExternalInput"
    ).ap()
    blv = nc.dram_tensor("blv", [L, 1, E], dt.bfloat16, kind="ExternalInput").ap()
    gam = nc.dram_tensor("gam", [L, P, KE], dt.float32, kind="ExternalInput").ap()
    bet = nc.dram_tensor("bet", [L, P, KE], dt.float32, kind="ExternalInput").ap()
    wo = nc.dram_tensor("wo", [KD, P, 2], dt.bfloat16, kind="ExternalInput").ap()
    bo = nc.dram_tensor("bo", [2, 1], dt.float32, kind="ExternalInput").ap()
    outT = nc.dram_tensor("outT", [ITEMS, 2, N], dt.float32, kind="ExternalOutput").ap()

    def chv(ap):
        # [.., NCH*CH] -> [.., NCH, CH] view so shapes line up with psum tiles
        return ap.rearrange("p (c n) -> p c n", c=NCH)

    with tile.TileContext(nc) as tc, ExitStack() as ctx:
        const = ctx.enter_context(tc.tile_pool(name="const", bufs=1))
        xpool = ctx.enter_context(tc.tile_pool(name="xres", bufs=1))
        wpool = ctx.enter_context(tc.tile_pool(name="wpool", bufs=2))
        lscal = ctx.enter_context(tc.tile_pool(name="lscal", bufs=2))
        aggp = ctx.enter_context(tc.tile_pool(name="aggp", bufs=4))
        tsqp = ctx.enter_context(tc.tile_pool(name="tsqp", bufs=1))
        zpool = ctx.enter_context(tc.tile_pool(name="zpool", bufs=2))
        bcp = ctx.enter_context(tc.tile_pool(name="bcp", bufs=2))
        smp = ctx.enter_context(tc.tile_pool(name="smp", bufs=2))
        obp = ctx.enter_context(tc.tile_pool(name="obp", bufs=2))
        pz = ctx.enter_context(tc.tile_pool(name="pz", bufs=6, space="PSUM"))
        pst = ctx.enter_context(tc.tile_pool(name="pst", bufs=2, space="PSUM"))

        ones_col = const.tile([P, 1], dt.bfloat16)
        nc.vector.memset(ones_col[:], 1.0)
        ones_row = const.tile([1, P], dt.bfloat16)
        nc.vector.memset(ones_row[:], 1.0)
        ones_n = const.tile([1, CH], dt.bfloat16)
        nc.vector.memset(ones_n[:], 1.0)
        eps_sb = const.tile([1, 1], dt.float32)
        nc.vector.memset(eps_sb[:], LN_EPS)
        bo_sb = const.tile([2, 1], dt.float32)
        nc.sync.dma_start(bo_sb[:], bo)
        wo_sb = const.tile([P, KD, 2], dt.bfloat16)
        nc.sync.dma_start(wo_sb[:], wo.rearrange("k p c -> p k c"))

        # Residual stream, resident for all 8 items: [P, item, d_chunk, node]
        x = xpool.tile([P, ITEMS, KD, NP], dt.bfloat16)
        nc.vector.memset(x[:], 0.0)

        def bcast(src_sb, lhs, nm):
            """[1, N] stat -> [P, N] psum (per-chunk one-bank tiles)."""
            tiles = []
            off = 0
            for ci, w in enumerate(SCH):
                bp = pz.tile([P, 512], dt.float32, tag="zps", name=f"b{nm}_{ci}")
                nc.tensor.matmul(
                    bp[:, 0:w],
                    lhsT=lhs[0:1, :],
                    rhs=src_sb[0:1, off : off + w],
                    start=True,
                    stop=True,
                )
                tiles.append(bp)
                off += w
            return tiles

        def ps_to_sb(out_sb, tiles, eng):
            off = 0
            for bp, w in zip(tiles, SCH):
                eng(out_sb[:, off : off + w], bp[:, 0:w])
                off += w

        def partition_reduce(rhs3, pool_tag):
            """sum over all 1024 partitions of [P, KD, N] -> two [1, w] psum tiles."""
            tiles = []
            off = 0
            for w in SCH:
                sp = pst.tile([1, 512], dt.float32, tag=pool_tag)
                for k in range(KD):
                    nc.tensor.matmul(
                        sp[0:1, 0:w],
                        lhsT=ones_col[:, 0:1],
                        rhs=rhs3[:, k, off : off + w],
                        start=(k == 0),
                        stop=(k == KD - 1),
                    )
                tiles.append(sp)
                off += w
            return tiles

        def stat_scale(out_sb, tiles):
            """[1,N] <- psum chunk tiles * (1/D)"""
            off = 0
            for sp, w in zip(tiles, SCH):
                nc.vector.tensor_scalar_mul(
                    out_sb[0:1, off : off + w], sp[0:1, 0:w], 1.0 / D
                )
                off += w

        # ---- software pipeline ----
        # Each item's LayerNorm stat chain is split into 7 stages; stages of
        # older items are emitted at "interleave points" placed between
        # e-tile pairs of the *current* item's main matmul stream, so the
        # in-order PE always has dense matmul work between dependent stat
        # pieces of earlier items.
        from collections import deque

        pending = deque()  # deque of per-item stage deques (oldest first)

        def point():
            for sl in list(pending):
                if sl:
                    sl.popleft()()
                if not sl:
                    pending.remove(sl)

        def make_stages(it, z_sb, ga_sb, be_sb):
            st = {}

            def t1():  # sum over features
                st["sum"] = partition_reduce(z_sb[:], "st")

            def t2():  # mu + broadcast matmuls
                mu_sb = smp.tile([1, N], dt.bfloat16, tag="mu")
                stat_scale(mu_sb, st["sum"])
                st["mub_ps"] = bcast(mu_sb, ones_row, f"mu{it}")

            def t3():  # mu_b to SBUF, center + square (chunked per k)
                mu_b = bcp.tile([P, N], dt.bfloat16, tag="mub")
                ps_to_sb(mu_b, st["mub_ps"], nc.scalar.copy)
                t = aggp.tile([P, KD, N], dt.bfloat16, tag="agg")
                tsq = tsqp.tile([P, KD, N], dt.bfloat16, tag="tsq")
                for k in range(KD):
                    nc.vector.tensor_sub(t[:, k, :], z_sb[:, k, :], mu_b[:])
                    nc.scalar.activation(tsq[:, k, :], t[:, k, :], F.Square)
                st["t"] = t
                st["tsq"] = tsq

            def t4():  # var-sum reduce of t^2
                st["sq"] = partition_reduce(st["tsq"][:], "st")

            def t5():  # var, rstd = exp(-0.5 ln(var+eps))
                var_sb = smp.tile([1, N], dt.float32, tag="var")
                stat_scale(var_sb, st["sq"])
                lnv_sb = smp.tile([1, N], dt.float32, tag="lnv")
                nc.scalar.activation(
                    lnv_sb[:], var_sb[:], F.Ln, bias=eps_sb[0:1, 0:1]
                )
                rstd_sb = smp.tile([1, N], dt.bfloat16, tag="rstd")
                nc.scalar.activation(rstd_sb[:], lnv_sb[:], F.Exp, scale=-0.5)
                st["rstd"] = rstd_sb

            def t6():  # rstd broadcast + normalize (in-place on GpSimd)
                rsb_ps = bcast(st["rstd"], ones_row, f"rs{it}")
                rstd_b = bcp.tile([P, N], dt.bfloat16, tag="rsb")
                ps_to_sb(rstd_b, rsb_ps, nc.scalar.copy)
                t = st["t"]
                nc.gpsimd.tensor_mul(
                    t[:], t[:], rstd_b[:, None, :].to_broadcast((P, KD, N))
                )

            def t7():  # gelu(gamma * t + beta) -> x
                for ke in range(KE):
                    nc.scalar.activation(
                        x[:, it, ke, COL0 : COL0 + N],
                        st["t"][:, ke, :],
                        F.Gelu,
                        bias=be_sb[:, ke : ke + 1],
                        scale=ga_sb[:, ke : ke + 1],
                    )

            return deque([t1, t2, t3, t4, t5, t6, t7])

        w_tiles = {}

        def load_w(l):
            w_tiles[l] = wpool.tile([P, KD, E], dt.bfloat16, tag="w", name=f"w_{l}")
            nc.sync.dma_start(w_tiles[l][:], wts[l].rearrange("k p e -> p k e"))

        def emit_agg(l, it, slot):
            """Build the matmul rhs for (l, it): adjacency aggregate for GCN
            layers, DMA'd features for the input projection.  Called one slot
            ahead so GpSimd/DVE finish before the PE needs it."""
            if l > 0:
                agg = aggp.tile([P, KD, N], dt.bfloat16, tag="agg", name=f"agg_{slot}")
                # shifted neighbor sum (1x mode: misaligned): split GpSimd/DVE
                for k in range(KD):
                    eng = nc.gpsimd if k < KD // 2 else nc.vector
                    eng.tensor_tensor(
                        agg[:, k, :],
                        x[:, it, k, COL0 - 1 : COL0 - 1 + N],
                        x[:, it, k, COL0 + 1 : COL0 + 1 + N],
                        op=OP.add,
                    )
                # center add (aligned, 2x mode) on VectorE, chunked per k
                for k in range(KD):
                    nc.vector.tensor_tensor(
                        agg[:, k, :],
                        agg[:, k, :],
                        x[:, it, k, COL0 : COL0 + N],
                        op=OP.add,
                    )
                return agg, None
            agg = aggp.tile([P, KD, N], dt.bfloat16, tag="agg", name=f"agg_{slot}")
            nc.gpsimd.dma_start(agg[:], featT[it].rearrange("k p n -> p k n"))
            pb_sb = zpool.tile([P, KD, N], dt.bfloat16, tag="z", name=f"pb_{slot}")
            nc.gpsimd.dma_start(pb_sb[:], posb[it].rearrange("k p n -> p k n"))
            return agg, pb_sb

        plan = [(l, it) for l in range(L + 1) for it in range(ITEMS)]
        load_w(0)
        layer_params = {}
        agg_next = emit_agg(*plan[0], 0)

        for j, (l, it) in enumerate(plan):
            if it == 0 and l > 0 and l not in layer_params:
                bl_sb = lscal.tile([1, E], dt.bfloat16, tag="bl", name=f"bl_{l}")
                nc.sync.dma_start(bl_sb[:], blv[l - 1])
                ga_sb = lscal.tile([P, KE], dt.float32, tag="ga", name=f"ga_{l}")
                nc.sync.dma_start(ga_sb[:], gam[l - 1])
                be_sb = lscal.tile([P, KE], dt.float32, tag="be", name=f"be_{l}")
                nc.sync.dma_start(be_sb[:], bet[l - 1])
                layer_params[l] = (bl_sb, ga_sb, be_sb)
            if l > 0:
                bl_sb, ga_sb, be_sb = layer_params[l]
            if it == 0:
                w_sb = w_tiles.pop(l)
            if it == 2 and l < L:
                load_w(l + 1)  # prefetch next layer's weights mid-layer

            agg, pb_sb = agg_next
            # prefetch next slot's rhs (GpSimd shift runs during this slot)
            agg_next = emit_agg(*plan[j + 1], j + 1) if j + 1 < len(plan) else None

            if l > 0:
                z_sb = zpool.tile([P, KD, N], dt.bfloat16, tag="z", name=f"z_{j}")
            else:
                z_sb = None

            if True:
                for ke in range(KE):
                    for c in range(NCH):
                        zps = pz.tile(
                            [P, 512], dt.float32, tag="zps", name=f"zps_{j}_{ke}_{c}"
                        )
                        for k in range(KD):
                            nc.tensor.matmul(
                                zps[:, 0:CH],
                                lhsT=w_sb[:, k, ke * P : (ke + 1) * P],
                                rhs=agg[:, k, c * CH : (c + 1) * CH],
                                start=(k == 0),
                                stop=(l == 0 and k == KD - 1),
                            )
                        if l > 0:
                            # += bl[e] via rank-1 matmul (K=1, rhs=ones)
                            nc.tensor.matmul(
                                zps[:, 0:CH],
                                lhsT=bl_sb[0:1, ke * P : (ke + 1) * P],
                                rhs=ones_n[0:1, 0:CH],
                                start=False,
                                stop=True,
                            )
                        dst = (
                            x[:, it, ke, COL0 + c * CH : COL0 + (c + 1) * CH]
                            if l == 0
                            else z_sb[:, ke, c * CH : (c + 1) * CH]
                        )
                        other = (
                            pb_sb[:, ke, c * CH : (c + 1) * CH]
                            if l == 0
                            else x[:, it, ke, COL0 + c * CH : COL0 + (c + 1) * CH]
                        )
                        nc.vector.tensor_tensor(
                            dst, zps[:, 0:CH], other, op=OP.add
                        )
                    if ke % 2 == 1:
                        point()

                if l > 0:
                    pending.append(make_stages(it, z_sb, ga_sb, be_sb))

        # output head: coords.T = Wo.T @ x -> [2, 600] per item, + bo.
        # Head matmuls double as PE filler while the stage pipeline drains.
        def head_chunk(it, c, ob):
            cps = pz.tile([P, 512], dt.float32, tag="zps", name=f"cps_{it}_{c}")
            for k in range(KD):
                nc.tensor.matmul(
                    cps[0:2, 0:CH],
                    lhsT=wo_sb[:, k, :],
                    rhs=x[:, it, k, COL0 + c * CH : COL0 + (c + 1) * CH],
                    start=(k == 0),
                    stop=(k == KD - 1),
                )
            nc.scalar.activation(
                ob[:, c * CH : (c + 1) * CH],
                cps[0:2, 0:CH],
                F.Identity,
                bias=bo_sb[:, 0:1],
            )
            if c == NCH - 1:
                nc.sync.dma_start(outT[it], ob[:])

        head_work = deque()
        for it in range(ITEMS):
            ob = obp.tile([2, N], dt.float32, tag="ob", name=f"ob_{it}")
            for c in range(NCH):
                head_work.append((it, c, ob))

        while pending or head_work:
            # head(it) needs x[it] final: its T7 ran two slots earlier, so
            # drain items 0..5 are ready immediately; 6/7 gate on their stages
            if head_work:
                it, c, ob = head_work.popleft()
                head_chunk(it, c, ob)
            point()

    nc.compile()
    return nc


def _get_nc():
    if "nc" not in _CACHE:
        _CACHE["nc"] = _build_nc()
    return _CACHE["nc"]


def _prep_inputs(features, positions, Wp, bp, pos_tab, Wl, bl, gamma, beta, Wo, bo):
    """Host-side packing: transpose/cast to the device layouts."""
    features = np.ascontiguousarray(np.asarray(features, np.float32))
    positions = np.asarray(positions)
    Wp = np.asarray(Wp, np.float32)
    bp = np.asarray(bp, np.float32)
    pos_tab = np.asarray(pos_tab, np.float32)
    Wl = np.asarray(Wl, np.float32)
    bl = np.asarray(bl, np.float32)
    gamma = np.asarray(gamma, np.float32)
    beta = np.asarray(beta, np.float32)
    Wo = np.asarray(Wo, np.float32)
    bo = np.asarray(bo, np.float32)

    featT = (
        features.transpose(0, 2, 1).reshape(B, KD, P, N).astype(BF16)
    )  # [B, k, p, n]
    # bp + pos_tab[positions]: [B, n, e] -> transposed/bf16 per item
    pe = pos_tab[positions] + bp[None, None, :]
    posbT = pe.transpose(0, 2, 1).reshape(B, KE, P, N).astype(BF16)

    wts = np.concatenate([Wp[None], Wl], axis=0)  # [L+1, d, e]
    wts = wts.reshape(L + 1, KD, P, E).astype(BF16)
    blv = bl.reshape(L, 1, E).astype(BF16)
    gam = np.ascontiguousarray(gamma.reshape(L, KE, P).transpose(0, 2, 1))  # [L, P, KE]
    bet = np.ascontiguousarray(beta.reshape(L, KE, P).transpose(0, 2, 1))
    woT = Wo.reshape(KD, P, 2).astype(BF16)
    bov = bo.reshape(2, 1)

    in_maps = []
    for c in range(NCORES):
        sl = slice(c * ITEMS, (c + 1) * ITEMS)
        in_maps.append(
            {
                "featT": np.ascontiguousarray(featT[sl]),
                "posb": np.ascontiguousarray(posbT[sl]),
                "wts": wts,
                "blv": blv,
                "gam": gam,
                "bet": bet,
                "wo": woT,
                "bo": bov,
            }
        )
    return in_maps


def run_device(in_maps, trace=False, **kwargs):
    """Compile (cached) and run the SPMD kernel; returns BassKernelResults."""
    from concourse import bass_utils

    nc = _get_nc()
    res = bass_utils.run_bass_kernel_spmd(
        nc, in_maps, core_ids=list(range(NCORES)), trace=trace, **kwargs
    )
    return res


def kernel(**inputs) -> np.ndarray:
    in_maps = _prep_inputs(
        inputs["features"],
        inputs["positions"],
        inputs["Wp"],
        inputs["bp"],
        inputs["pos_tab"],
        inputs["Wl"],
        inputs["bl"],
        inputs["gamma"],
        inputs["beta"],
        inputs["Wo"],
        inputs["bo"],
    )
    res = run_device(in_maps, trace=False)
    out = np.empty((B, 600, 2), np.float32)
    for c in range(NCORES):
        o = res.results[c]["outT"]  # [ITEMS, 2, N]
        out[c * ITEMS : (c + 1) * ITEMS] = o.transpose(0, 2, 1)
    out[:, 0, :] = [0.0, 0.0]
    out[:, -1, :] = [600.0, 0.0]
    return out


# revision 30
# speedup vs baseline: 22.5188x; 1.5830x over previous
"""Trainium2 Bass kernel for nn_CoordinateGCN (8-layer GCN, tridiagonal adjacency).

Strategy
--------
Pure data parallel over the batch: 64 items -> 8 NeuronCores x 8 items.
On-chip layout is feature-major ("transposed"): activations live in SBUF as
x[d, n] with the 1024-dim feature axis on partitions (8 chunks of 128) and the
600 nodes on the free axis.  Every matmul is then native:
    z[e, n] += W[d_chunk, e_tile].T @ rhs[d_chunk, n]     (PSUM fp32 accumulate)
The tridiagonal adjacency is two shifted elementwise adds (never a matmul).
LayerNorm runs in this layout via ones-matmul partition reductions and
ones-outer-product broadcasts; variance is computed from the centered tensor
(var = mean(t^2), t = z - mu_broadcast) which the LN apply needs anyway.
gelu(gamma*t+beta) is a single ScalarE op per e-tile (gamma/beta are
per-partition scalars).  All matmul operands are bf16 (1 cycle/row on the PE
vs 4 for fp32) with fp32 PSUM accumulation.

The host wrapper transposes/casts inputs, folds bp + pos_tab[positions] into a
per-item additive table, and re-assembles the full fp32 output.
"""

import sys

sys.path.insert(0, "/opt/trn_rl_repo")

import numpy as np
import ml_dtypes

BF16 = ml_dtypes.bfloat16

# Problem shapes (hardcoded per the harness contract).
B = 64
NCORES = 8
ITEMS = B // NCORES
P = 128
D = 1024  # input dim == embed dim
KD = D // P
E = 1024
KE = E // P
N = 600
NP = 604  # padded node columns; data at [2, 602), zeros at cols 1 and 602
COL0 = 2
L = 8
CH = 300
NCH = 2
SCH = (512, 88)  # stats chunking: each chunk stays inside one PSUM bank
LN_EPS = 1e-5

_CACHE = {}


def _build_nc():
    from contextlib import ExitStack

    import concourse.bass as bass  # noqa: F401
    import concourse.tile as tile
    from concourse import bacc
    import concourse.mybir as mybir

    dt = mybir.dt
    F = mybir.ActivationFunctionType
    OP = mybir.AluOpType

    nc = bacc.Bacc("TRN2", target_bir_lowering=False, debug=False, num_devices=NCORES)

    featT = nc.dram_tensor(
        "featT", [ITEMS, KD, P, N], dt.bfloat16, kind="ExternalInput"
    ).ap()
    posb = nc.dram_tensor(
        "posb", [ITEMS, KE, P, N], dt.bfloat16, kind="ExternalInput"
    ).ap()
    # wts[0] = Wp (input projection), wts[1..L] = per-layer GCN weights
    wts = nc.dram_tensor(
        "wts", [L + 1, KD, P, E], dt.bfloat16, kind="ExternalInput"
    ).ap()
    blv = nc.dram_tensor("blv", [L, 1, E], dt.bfloat16, kind="ExternalInput").ap()
    gam = nc.dram_tensor("gam", [L, P, KE], dt.float32, kind="ExternalInput").ap()
    bet = nc.dram_tensor("bet", [L, P, KE], dt.float32, kind="ExternalInput").ap()
    wo = nc.dram_tensor("wo", [KD, P, 2], dt.bfloat16, kind="ExternalInput").ap()
    bo = nc.dram_tensor("bo", [2, 1], dt.float32, kind="ExternalInput").ap()
    outT = nc.dram_tensor("outT", [ITEMS, 2, N], dt.float32, kind="ExternalOutput").ap()

    def chv(ap):
        # [.., NCH*CH] -> [.., NCH, CH] view so shapes line up with psum tiles
        return ap.rearrange("p (c n) -> p c n", c=NCH)

    with tile.TileContext(nc) as tc, ExitStack() as ctx:
        const = ctx.enter_context(tc.tile_pool(name="const", bufs=1))
        xpool = ctx.enter_context(tc.tile_pool(name="xres", bufs=1))
        wpool = ctx.enter_context(tc.tile_pool(name="wpool", bufs=2))
        lscal = ctx.enter_context(tc.tile_pool(name="lscal", bufs=2))
        aggp = ctx.enter_context(tc.tile_pool(name="aggp", bufs=4))
        tsqp = ctx.enter_context(tc.tile_pool(name="tsqp", bufs=1))
        zpool = ctx.enter_context(tc.tile_pool(name="zpool", bufs=2))
        bcp = ctx.enter_context(tc.tile_pool(name="bcp", bufs=2))
        smp = ctx.enter_context(tc.tile_pool(name="smp", bufs=2))
        obp = ctx.enter_context(tc.tile_pool(name="obp", bufs=2))
        pz = ctx.enter_context(tc.tile_pool(name="pz", bufs=6, space="PSUM"))
        pst = ctx.enter_context(tc.tile_pool(name="pst", bufs=2, space="PSUM"))

        ones_col = const.tile([P, 1], dt.bfloat16)
        nc.vector.memset(ones_col[:], 1.0)
        ones_row = const.tile([1, P], dt.bfloat16)
        nc.vector.memset(ones_row[:], 1.0)
        ones_n = const.tile([1, CH], dt.bfloat16)
        nc.vector.memset(ones_n[:], 1.0)
        eps_sb = const.tile([1, 1], dt.float32)
        nc.vector.memset(eps_sb[:], LN_EPS)
        bo_sb = const.tile([2, 1], dt.float32)
        nc.sync.dma_start(bo_sb[:], bo)
        wo_sb = const.tile([P, KD, 2], dt.bfloat16)
        nc.sync.dma_start(wo_sb[:], wo.rearrange("k p c -> p k c"))

        # Residual stream, resident for all 8 items: [P, item, d_chunk, node]
        x = xpool.tile([P, ITEMS, KD, NP], dt.bfloat16)
        nc.vector.memset(x[:], 0.0)

        def bcast(src_sb, lhs, nm):
            """[1, N] stat -> [P, N] psum (per-chunk one-bank tiles)."""
            tiles = []
            off = 0
            for ci, w in enumerate(SCH):
                bp = pz.tile([P, 512], dt.float32, tag="zps", name=f"b{nm}_{ci}")
                nc.tensor.matmul(
                    bp[:, 0:w],
                    lhsT=lhs[0:1, :],
                    rhs=src_sb[0:1, off : off + w],
                    start=True,
                    stop=True,
                )
                tiles.append(bp)
                off += w
            return tiles

        def ps_to_sb(out_sb, tiles, eng):
            off = 0
            for bp, w in zip(tiles, SCH):
                eng(out_sb[:, off : off + w], bp[:, 0:w])
                off += w

        def partition_reduce(rhs3, pool_tag):
            """sum over all 1024 partitions of [P, KD, N] -> two [1, w] psum tiles."""
            tiles = []
            off = 0
            for w in SCH:
                sp = pst.tile([1, 512], dt.float32, tag=pool_tag)
                for k in range(KD):
                    nc.tensor.matmul(
                        sp[0:1, 0:w],
                        lhsT=ones_col[:, 0:1],
                        rhs=rhs3[:, k, off : off + w],
                        start=(k == 0),
                        stop=(k == KD - 1),
                    )
                tiles.append(sp)
                off += w
            return tiles

        def stat_scale(out_sb, tiles):
            """[1,N] <- psum chunk tiles * (1/D)"""
            off = 0
            for sp, w in zip(tiles, SCH):
                nc.vector.tensor_scalar_mul(
                    out_sb[0:1, off : off + w], sp[0:1, 0:w], 1.0 / D
                )
                off += w

        # ---- software pipeline ----
        # Each item's LayerNorm stat chain is split into 7 stages; stages of
        # older items are emitted at "interleave points" placed between
        # e-tile pairs of the *current* item's main matmul stream, so the
        # in-order PE always has dense matmul work between dependent stat
        # pieces of earlier items.
        from collections import deque

        pending = deque()  # deque of per-item stage deques (oldest first)

        def point():
            for sl in list(pending):
                if sl:
                    sl.popleft()()
                if not sl:
                    pending.remove(sl)

        def make_stages(it, z_sb, ga_sb, be_sb):
            st = {}

            def t1():  # sum over features
                st["sum"] = partition_reduce(z_sb[:], "st")

            def t2():  # mu + broadcast matmuls
                mu_sb = smp.tile([1, N], dt.bfloat16, tag="mu")
                stat_scale(mu_sb, st["sum"])
                st["mub_ps"] = bcast(mu_sb, ones_row, f"mu{it}")

            def t3():  # mu_b to SBUF, center + square (chunked per k)
                mu_b = bcp.tile([P, N], dt.bfloat16, tag="mub")
                ps_to_sb(mu_b, st["mub_ps"], nc.scalar.copy)
                t = aggp.tile([P, KD, N], dt.bfloat16, tag="agg")
                tsq = tsqp.tile([P, KD, N], dt.bfloat16, tag="tsq")
                for k in range(KD):
                    nc.vector.tensor_sub(t[:, k, :], z_sb[:, k, :], mu_b[:])
                    nc.scalar.activation(tsq[:, k, :], t[:, k, :], F.Square)
                st["t"] = t
                st["tsq"] = tsq

            def t4():  # var-sum reduce of t^2
                st["sq"] = partition_reduce(st["tsq"][:], "st")

            def t5():  # var, rstd = exp(-0.5 ln(var+eps))
                var_sb = smp.tile([1, N], dt.float32, tag="var")
                stat_scale(var_sb, st["sq"])
                lnv_sb = smp.tile([1, N], dt.float32, tag="lnv")
                nc.scalar.activation(
                    lnv_sb[:], var_sb[:], F.Ln, bias=eps_sb[0:1, 0:1]
                )
                rstd_sb = smp.tile([1, N], dt.bfloat16, tag="rstd")
                nc.scalar.activation(rstd_sb[:], lnv_sb[:], F.Exp, scale=-0.5)
                st["rstd"] = rstd_sb

            def t6():  # rstd broadcast + normalize (in-place on GpSimd)
                rsb_ps = bcast(st["rstd"], ones_row, f"rs{it}")
                rstd_b = bcp.tile([P, N], dt.bfloat16, tag="rsb")
                ps_to_sb(rstd_b, rsb_ps, nc.scalar.copy)
                t = st["t"]
                nc.gpsimd.tensor_mul(
                    t[:], t[:], rstd_b[:, None, :].to_broadcast((P, KD, N))
                )

            def t7():  # gelu(gamma * t + beta) -> x
                for ke in range(KE):
                    nc.scalar.activation(
                        x[:, it, ke, COL0 : COL0 + N],
                        st["t"][:, ke, :],
                        F.Gelu,
                        bias=be_sb[:, ke : ke + 1],
                        scale=ga_sb[:, ke : ke + 1],
                    )

            return deque([t1, t2, t3, t4, t5, t6, t7])

        w_tiles = {}

        def load_w(l):
            w_tiles[l] = wpool.tile([P, KD, E], dt.bfloat16, tag="w", name=f"w_{l}")
            nc.sync.dma_start(w_tiles[l][:], wts[l].rearrange("k p e -> p k e"))

        def emit_agg(l, it, slot):
            """Build the matmul rhs for (l, it): adjacency aggregate for GCN
            layers, DMA'd features for the input projection.  Called one slot
            ahead so GpSimd/DVE finish before the PE needs it."""
            if l > 0:
                agg = aggp.tile([P, KD, N], dt.bfloat16, tag="agg", name=f"agg_{slot}")
                # shifted neighbor sum (1x mode: misaligned): split GpSimd/DVE
                for k in range(KD):
                    eng = nc.gpsimd if k < KD // 2 else nc.vector
                    eng.tensor_tensor(
                        agg[:, k, :],
                        x[:, it, k, COL0 - 1 : COL0 - 1 + N],
                        x[:, it, k, COL0 + 1 : COL0 + 1 + N],
                        op=OP.add,
                    )
                # center add (aligned, 2x mode) on VectorE, chunked per k
                for k in range(KD):
                    nc.vector.tensor_tensor(
                        agg[:, k, :],
                        agg[:, k, :],
                        x[:, it, k, COL0 : COL0 + N],
                        op=OP.add,
                    )
                return agg, None
            agg = aggp.tile([P, KD, N], dt.bfloat16, tag="agg", name=f"agg_{slot}")
            nc.gpsimd.dma_start(agg[:], featT[it].rearrange("k p n -> p k n"))
            pb_sb = zpool.tile([P, KD, N], dt.bfloat16, tag="z", name=f"pb_{slot}")
            nc.gpsimd.dma_start(pb_sb[:], posb[it].rearrange("k p n -> p k n"))
            return agg, pb_sb

        plan = [(l, it) for l in range(L + 1) for it in range(ITEMS)]
        load_w(0)
        layer_params = {}
        agg_next = emit_agg(*plan[0], 0)

        for j, (l, it) in enumerate(plan):
            if it == 0 and l > 0 and l not in layer_params:
                bl_sb = lscal.tile([1, E], dt.bfloat16, tag="bl", name=f"bl_{l}")
                nc.sync.dma_start(bl_sb[:], blv[l - 1])
                ga_sb = lscal.tile([P, KE], dt.float32, tag="ga", name=f"ga_{l}")
                nc.sync.dma_start(ga_sb[:], gam[l - 1])
                be_sb = lscal.tile([P, KE], dt.float32, tag="be", name=f"be_{l}")
                nc.sync.dma_start(be_sb[:], bet[l - 1])
                layer_params[l] = (bl_sb, ga_sb, be_sb)
            if l > 0:
                bl_sb, ga_sb, be_sb = layer_params[l]
            if it == 0:
                w_sb = w_tiles.pop(l)
            if it == 2 and l < L:
                load_w(l + 1)  # prefetch next layer's weights mid-layer

            agg, pb_sb = agg_next
            # prefetch next slot's rhs (GpSimd shift runs during this slot)
            agg_next = emit_agg(*plan[j + 1], j + 1) if j + 1 < len(plan) else None

            if l > 0:
                z_sb = zpool.tile([P, KD, N], dt.bfloat16, tag="z", name=f"z_{j}")
            else:
                z_sb = None

            if True:
                for ke in range(KE):
                    for c in range(NCH):
                        zps = pz.tile(
                            [P, 512], dt.float32, tag="zps", name=f"zps_{j}_{ke}_{c}"
                        )
                        for k in range(KD):
                            nc.tensor.matmul(
                                zps[:, 0:CH],
                                lhsT=w_sb[:, k, ke * P : (ke + 1) * P],
                                rhs=agg[:, k, c * CH : (c + 1) * CH],
                                start=(k == 0),
                                stop=(l == 0 and k == KD - 1),
                            )
                        if l > 0:
                            # += bl[e] via rank-1 matmul (K=1, rhs=ones)
                            nc.tensor.matmul(
                                zps[:, 0:CH],
                                lhsT=bl_sb[0:1, ke * P : (ke + 1) * P],
                                rhs=ones_n[0:1, 0:CH],
                                start=False,
                                stop=True,
                            )
                        dst = (
                            x[:, it, ke, COL0 + c * CH : COL0 + (c + 1) * CH]
                            if l == 0
                            else z_sb[:, ke, c * CH : (c + 1) * CH]
                        )
                        other = (
                            pb_sb[:, ke, c * CH : (c + 1) * CH]
                            if l == 0
                            else x[:, it, ke, COL0 + c * CH : COL0 + (c + 1) * CH]
                        )
                        nc.vector.tensor_tensor(
                            dst, zps[:, 0:CH], other, op=OP.add
                        )
                    if ke % 2 == 1:
                        point()

                if l > 0:
                    pending.append(make_stages(it, z_sb, ga_sb, be_sb))

        # output head: coords.T = Wo.T @ x -> [2, 600] per item, + bo.
        # Head matmuls double as PE filler while the stage pipeline drains.
        def head_chunk(it, c, ob):
            cps = pz.tile([P, 512], dt.float32, tag="zps", name=f"cps_{it}_{c}")
            for k in range(KD):
                nc.tensor.matmul(
                    cps[0:2, 0:CH],
                    lhsT=wo_sb[:, k, :],
                    rhs=x[:, it, k, COL0 + c * CH : COL0 + (c + 1) * CH],
                    start=(k == 0),
                    stop=(k == KD - 1),
                )
            nc.scalar.activation(
                ob[:, c * CH : (c + 1) * CH],
                cps[0:2, 0:CH],
                F.Identity,
                bias=bo_sb[:, 0:1],
            )
            if c == NCH - 1:
                nc.sync.dma_start(outT[it], ob[:])

        head_work = deque()
        for it in range(ITEMS):
            ob = obp.tile([2, N], dt.float32, tag="ob", name=f"ob_{it}")
            for c in range(NCH):
                head_work.append((it, c, ob))

        while pending or head_work:
            # head(it) needs x[it] final: its T7 ran two slots earlier, so
            # drain items 0..5 are ready immediately; 6/7 gate on their stages
            if head_work:
                it, c, ob = head_work.popleft()
                head_chunk(it, c, ob)
            point()

    nc.compile()
    return nc


def _get_nc():
    if "nc" not in _CACHE:
        _CACHE["nc"] = _build_nc()
    return _CACHE["nc"]


def _prep_inputs(features, positions, Wp, bp, pos_tab, Wl, bl, gamma, beta, Wo, bo):
    """Host-side packing: transpose/cast to the device layouts."""
    features = np.ascontiguousarray(np.asarray(features, np.float32))
    positions = np.asarray(positions)
    Wp = np.asarray(Wp, np.float32)
    bp = np.asarray(bp, np.float32)
    pos_tab = np.asarray(pos_tab, np.float32)
    Wl = np.asarray(Wl, np.float32)
    bl = np.asarray(bl, np.float32)
    gamma = np.asarray(gamma, np.float32)
    beta = np.asarray(beta, np.float32)
    Wo = np.asarray(Wo, np.float32)
    bo = np.asarray(bo, np.float32)

    featT = (
        features.transpose(0, 2, 1).reshape(B, KD, P, N).astype(BF16)
    )  # [B, k, p, n]
    # bp + pos_tab[positions]: [B, n, e] -> transposed/bf16 per item
    pe = pos_tab[positions] + bp[None, None, :]
    posbT = pe.transpose(0, 2, 1).reshape(B, KE, P, N).astype(BF16)

    wts = np.concatenate([Wp[None], Wl], axis=0)  # [L+1, d, e]
    wts = wts.reshape(L + 1, KD, P, E).astype(BF16)
    blv = bl.reshape(L, 1, E).astype(BF16)
    gam = np.ascontiguousarray(gamma.reshape(L, KE, P).transpose(0, 2, 1))  # [L, P, KE]
    bet = np.ascontiguousarray(beta.reshape(L, KE, P).transpose(0, 2, 1))
    woT = Wo.reshape(KD, P, 2).astype(BF16)
    bov = bo.reshape(2, 1)

    in_maps = []
    for c in range(NCORES):
        sl = slice(c * ITEMS, (c + 1) * ITEMS)
        in_maps.append(
            {
                "featT": np.ascontiguousarray(featT[sl]),
                "posb": np.ascontiguousarray(posbT[sl]),
                "wts": wts,
                "blv": blv,
                "gam": gam,
                "bet": bet,
                "wo": woT,
                "bo": bov,
            }
        )
    return in_maps


def run_device(in_maps, trace=False, **kwargs):
    """Compile (cached) and run the SPMD kernel; returns BassKernelResults."""
    from concourse import bass_utils

    nc = _get_nc()
    res = bass_utils.run_bass_kernel_spmd(
        nc, in_maps, core_ids=list(range(NCORES)), trace=trace, **kwargs
    )
    return res


def kernel(**inputs) -> np.ndarray:
    in_maps = _prep_inputs(
        inputs["features"],
        inputs["positions"],
        inputs["Wp"],
        inputs["bp"],
        inputs["pos_tab"],
        inputs["Wl"],
        inputs["bl"],
        inputs["gamma"],
        inputs["beta"],
        inputs["Wo"],
        inputs["bo"],
    )
    res = run_device(in_maps, trace=False)
    out = np.empty((B, 600, 2), np.float32)
    for c in range(NCORES):
        o = res.results[c]["outT"]  # [ITEMS, 2, N]
        out[c * ITEMS : (c + 1) * ITEMS] = o.transpose(0, 2, 1)
    out[:, 0, :] = [0.0, 0.0]
    out[:, -1, :] = [600.0, 0.0]
    return out


# revision 35
# speedup vs baseline: 27.8336x; 1.2360x over previous
"""Trainium2 Bass kernel for nn_CoordinateGCN (8-layer GCN, tridiagonal adjacency).

Strategy
--------
Pure data parallel over the batch: 64 items -> 8 NeuronCores x 8 items.
On-chip layout is feature-major ("transposed"): activations live in SBUF as
x[d, n] with the 1024-dim feature axis on partitions (8 chunks of 128) and the
600 nodes on the free axis.  Every matmul is then native:
    z[e, n] += W[d_chunk, e_tile].T @ rhs[d_chunk, n]     (PSUM fp32 accumulate)
The tridiagonal adjacency is two shifted elementwise adds (never a matmul).
LayerNorm runs in this layout via ones-matmul partition reductions and
ones-outer-product broadcasts; variance is computed from the centered tensor
(var = mean(t^2), t = z - mu_broadcast) which the LN apply needs anyway.
gelu(gamma*t+beta) is a single ScalarE op per e-tile (gamma/beta are
per-partition scalars).  All matmul operands are bf16 (1 cycle/row on the PE
vs 4 for fp32) with fp32 PSUM accumulation.

The host wrapper transposes/casts inputs, folds bp + pos_tab[positions] into a
per-item additive table, and re-assembles the full fp32 output.
"""

import sys

sys.path.insert(0, "/opt/trn_rl_repo")

import numpy as np
import ml_dtypes

BF16 = ml_dtypes.bfloat16

# Problem shapes (hardcoded per the harness contract).
B = 64
NCORES = 8
ITEMS = B // NCORES
P = 128
D = 1024  # input dim == embed dim
KD = D // P
E = 1024
KE = E // P
N = 600
NP = 604  # padded node columns; data at [2, 602), zeros at cols 1 and 602
COL0 = 2
L = 8
CH = 300
NCH = 2
SCH = (512, 88)  # stats chunking: each chunk stays inside one PSUM bank
LN_EPS = 1e-5

_CACHE = {}


def _build_nc():
    from contextlib import ExitStack

    import concourse.bass as bass  # noqa: F401
    import concourse.tile as tile
    from concourse import bacc
    import concourse.mybir as mybir

    dt = mybir.dt
    F = mybir.ActivationFunctionType
    OP = mybir.AluOpType

    nc = bacc.Bacc("TRN2", target_bir_lowering=False, debug=False, num_devices=NCORES)

    featT = nc.dram_tensor(
        "featT", [ITEMS, KD, P, N], dt.bfloat16, kind="ExternalInput"
    ).ap()
    posb = nc.dram_tensor(
        "posb", [ITEMS, KE, P, N], dt.bfloat16, kind="ExternalInput"
    ).ap()
    # wts[0] = Wp (input projection), wts[1..L] = per-layer GCN weights
    wts = nc.dram_tensor(
        "wts", [L + 1, KD, P, E], dt.bfloat16, kind="ExternalInput"
    ).ap()
    blv = nc.dram_tensor("blv", [L, 1, E], dt.bfloat16, kind="ExternalInput").ap()
    gam = nc.dram_tensor("gam", [L, P, KE], dt.float32, kind="ExternalInput").ap()
    bet = nc.dram_tensor("bet", [L, P, KE], dt.float32, kind="ExternalInput").ap()
    wo = nc.dram_tensor("wo", [KD, P, 2], dt.bfloat16, kind="ExternalInput").ap()
    bo = nc.dram_tensor("bo", [2, 1], dt.float32, kind="ExternalInput").ap()
    outT = nc.dram_tensor("outT", [ITEMS, 2, N], dt.float32, kind="ExternalOutput").ap()

    def chv(ap):
        # [.., NCH*CH] -> [.., NCH, CH] view so shapes line up with psum tiles
        return ap.rearrange("p (c n) -> p c n", c=NCH)

    with tile.TileContext(nc) as tc, ExitStack() as ctx:
        const = ctx.enter_context(tc.tile_pool(name="const", bufs=1))
        xpool = ctx.enter_context(tc.tile_pool(name="xres", bufs=1))
        wpool = ctx.enter_context(tc.tile_pool(name="wpool", bufs=2))
        lscal = ctx.enter_context(tc.tile_pool(name="lscal", bufs=2))
        aggp = ctx.enter_context(tc.tile_pool(name="aggp", bufs=4))
        tsqp = ctx.enter_context(tc.tile_pool(name="tsqp", bufs=1))
        zpool = ctx.enter_context(tc.tile_pool(name="zpool", bufs=2))
        bcp = ctx.enter_context(tc.tile_pool(name="bcp", bufs=2))
        smp = ctx.enter_context(tc.tile_pool(name="smp", bufs=2))
        obp = ctx.enter_context(tc.tile_pool(name="obp", bufs=2))
        pz = ctx.enter_context(tc.tile_pool(name="pz", bufs=4, space="PSUM"))
        pst = ctx.enter_context(tc.tile_pool(name="pst", bufs=4, space="PSUM"))

        ones_col = const.tile([P, 1], dt.bfloat16)
        nc.vector.memset(ones_col[:], 1.0)
        ones_row = const.tile([1, P], dt.bfloat16)
        nc.vector.memset(ones_row[:], 1.0)
        ones_n = const.tile([1, CH], dt.bfloat16)
        nc.vector.memset(ones_n[:], 1.0)
        eps_sb = const.tile([1, 1], dt.float32)
        nc.vector.memset(eps_sb[:], LN_EPS)
        bo_sb = const.tile([2, 1], dt.float32)
        nc.sync.dma_start(bo_sb[:], bo)
        wo_sb = const.tile([P, KD, 2], dt.bfloat16)
        nc.sync.dma_start(wo_sb[:], wo.rearrange("k p c -> p k c"))

        # Residual stream, resident for all 8 items: [P, item, d_chunk, node]
        x = xpool.tile([P, ITEMS, KD, NP], dt.bfloat16)
        nc.vector.memset(x[:], 0.0)

        def bcast(src_sb, lhs, nm):
            """[1, N] stat -> [P, N] psum (per-chunk one-bank tiles)."""
            tiles = []
            off = 0
            for ci, w in enumerate(SCH):
                bp = pz.tile([P, 512], dt.float32, tag="zps", name=f"b{nm}_{ci}")
                nc.tensor.matmul(
                    bp[:, 0:w],
                    lhsT=lhs[0:1, :],
                    rhs=src_sb[0:1, off : off + w],
                    start=True,
                    stop=True,
                )
                tiles.append(bp)
                off += w
            return tiles

        def ps_to_sb(out_sb, tiles, eng):
            off = 0
            for bp, w in zip(tiles, SCH):
                eng(out_sb[:, off : off + w], bp[:, 0:w])
                off += w

        def partition_reduce(rhs3, pool_tag):
            """sum over all 1024 partitions of [P, KD, N] -> two [1, w] psum tiles."""
            tiles = []
            off = 0
            for w in SCH:
                sp = pst.tile([1, 512], dt.float32, tag=pool_tag)
                for k in range(KD):
                    nc.tensor.matmul(
                        sp[0:1, 0:w],
                        lhsT=ones_col[:, 0:1],
                        rhs=rhs3[:, k, off : off + w],
                        start=(k == 0),
                        stop=(k == KD - 1),
                    )
                tiles.append(sp)
                off += w
            return tiles

        def stat_scale(out_sb, tiles):
            """[1,N] <- psum chunk tiles * (1/D)"""
            off = 0
            for sp, w in zip(tiles, SCH):
                nc.vector.tensor_scalar_mul(
                    out_sb[0:1, off : off + w], sp[0:1, 0:w], 1.0 / D
                )
                off += w

        # ---- software pipeline ----
        # Each item's LayerNorm stat chain is split into 7 stages; stages of
        # older items are emitted at "interleave points" placed between
        # e-tile pairs of the *current* item's main matmul stream, so the
        # in-order PE always has dense matmul work between dependent stat
        # pieces of earlier items.
        from collections import deque

        pending = deque()  # deque of per-item stage deques (oldest first)

        def point():
            # Pop one stage per pending item (oldest first), but emit the
            # newer item's T3 (center+square feeding imminent stats matmuls)
            # ahead of the older item's T7 gelus, which have slots of slack —
            # this keeps the urgent squares ahead in the ACT engine queue.
            todo = []
            for sl in list(pending):
                if sl:
                    todo.append(sl.popleft())
                if not sl:
                    pending.remove(sl)
            todo.sort(key=lambda f: f.__name__ == "t7")
            for f in todo:
                f()

        def make_stages(it, z_sb, ga_sb, be_sb):
            st = {}

            def t1():  # sum over features
                st["sum"] = partition_reduce(z_sb[:], "st")

            def t3():  # mu, broadcast, center + square — early, so the
                # sub->square relay finishes well before t4's matmuls
                mu_sb = smp.tile([1, N], dt.bfloat16, tag="mu")
                stat_scale(mu_sb, st["sum"])
                mub_ps = bcast(mu_sb, ones_row, f"mu{it}")
                mu_b = bcp.tile([P, N], dt.bfloat16, tag="mub")
                ps_to_sb(mu_b, mub_ps, nc.scalar.copy)
                t = aggp.tile([P, KD, N], dt.bfloat16, tag="agg")
                tsq = tsqp.tile([P, KD, N], dt.bfloat16, tag="tsq")
                for k in range(KD):
                    nc.vector.tensor_sub(t[:, k, :], z_sb[:, k, :], mu_b[:])
                    nc.scalar.activation(tsq[:, k, :], t[:, k, :], F.Square)
                st["t"] = t
                st["tsq"] = tsq

            def nop():
                pass

            def t4():  # var-sum reduce of t^2
                st["sq"] = partition_reduce(st["tsq"][:], "st")

            def t5():  # var, rstd = exp(-0.5 ln(var+eps))
                var_sb = smp.tile([1, N], dt.float32, tag="var")
                stat_scale(var_sb, st["sq"])
                lnv_sb = smp.tile([1, N], dt.float32, tag="lnv")
                nc.scalar.activation(
                    lnv_sb[:], var_sb[:], F.Ln, bias=eps_sb[0:1, 0:1]
                )
                rstd_sb = smp.tile([1, N], dt.bfloat16, tag="rstd")
                nc.scalar.activation(rstd_sb[:], lnv_sb[:], F.Exp, scale=-0.5)
                st["rstd"] = rstd_sb

            def t6():  # rstd broadcast + normalize (in-place on GpSimd)
                rsb_ps = bcast(st["rstd"], ones_row, f"rs{it}")
                rstd_b = bcp.tile([P, N], dt.bfloat16, tag="rsb")
                ps_to_sb(rstd_b, rsb_ps, nc.scalar.copy)
                t = st["t"]
                nc.gpsimd.tensor_mul(
                    t[:], t[:], rstd_b[:, None, :].to_broadcast((P, KD, N))
                )

            def t7():  # gelu(gamma * t + beta) -> x
                for ke in range(KE):
                    nc.scalar.activation(
                        x[:, it, ke, COL0 : COL0 + N],
                        st["t"][:, ke, :],
                        F.Gelu,
                        bias=be_sb[:, ke : ke + 1],
                        scale=ga_sb[:, ke : ke + 1],
                    )

            return deque([t1, t3, nop, t4, t5, t6, t7])

        w_tiles = {}

        def load_w(l):
            w_tiles[l] = wpool.tile([P, KD, E], dt.bfloat16, tag="w", name=f"w_{l}")
            nc.sync.dma_start(w_tiles[l][:], wts[l].rearrange("k p e -> p k e"))

        def emit_agg(l, it, slot):
            """Build the matmul rhs for (l, it): adjacency aggregate for GCN
            layers, DMA'd features for the input projection.  Called one slot
            ahead so GpSimd/DVE finish before the PE needs it."""
            if l > 0:
                agg = aggp.tile([P, KD, N], dt.bfloat16, tag="agg", name=f"agg_{slot}")
                # shifted neighbor sum (1x mode: misaligned): split GpSimd/DVE
                for k in range(KD):
                    eng = nc.gpsimd if k < KD // 2 else nc.vector
                    eng.tensor_tensor(
                        agg[:, k, :],
                        x[:, it, k, COL0 - 1 : COL0 - 1 + N],
                        x[:, it, k, COL0 + 1 : COL0 + 1 + N],
                        op=OP.add,
                    )
                # center add (aligned, 2x mode) on VectorE, chunked per k
                for k in range(KD):
                    nc.vector.tensor_tensor(
                        agg[:, k, :],
                        agg[:, k, :],
                        x[:, it, k, COL0 : COL0 + N],
                        op=OP.add,
                    )
                return agg, None
            agg = aggp.tile([P, KD, N], dt.bfloat16, tag="agg", name=f"agg_{slot}")
            nc.gpsimd.dma_start(agg[:], featT[it].rearrange("k p n -> p k n"))
            pb_sb = zpool.tile([P, KD, N], dt.bfloat16, tag="z", name=f"pb_{slot}")
            nc.gpsimd.dma_start(pb_sb[:], posb[it].rearrange("k p n -> p k n"))
            return agg, pb_sb

        plan = [(l, it) for l in range(L + 1) for it in range(ITEMS)]
        load_w(0)
        layer_params = {}
        agg_next = emit_agg(*plan[0], 0)

        for j, (l, it) in enumerate(plan):
            if it == 0 and l > 0 and l not in layer_params:
                bl_sb = lscal.tile([1, E], dt.bfloat16, tag="bl", name=f"bl_{l}")
                nc.sync.dma_start(bl_sb[:], blv[l - 1])
                ga_sb = lscal.tile([P, KE], dt.float32, tag="ga", name=f"ga_{l}")
                nc.sync.dma_start(ga_sb[:], gam[l - 1])
                be_sb = lscal.tile([P, KE], dt.float32, tag="be", name=f"be_{l}")
                nc.sync.dma_start(be_sb[:], bet[l - 1])
                layer_params[l] = (bl_sb, ga_sb, be_sb)
            if l > 0:
                bl_sb, ga_sb, be_sb = layer_params[l]
            if it == 0:
                w_sb = w_tiles.pop(l)
            if it == 2 and l < L:
                load_w(l + 1)  # prefetch next layer's weights mid-layer

            agg, pb_sb = agg_next
            # prefetch next slot's rhs (GpSimd shift runs during this slot)
            agg_next = emit_agg(*plan[j + 1], j + 1) if j + 1 < len(plan) else None

            if l > 0:
                z_sb = zpool.tile([P, KD, N], dt.bfloat16, tag="z", name=f"z_{j}")
            else:
                z_sb = None

            if True:
                for ke in range(KE):
                    for c in range(NCH):
                        zps = pz.tile(
                            [P, 512], dt.float32, tag="zps", name=f"zps_{j}_{ke}_{c}"
                        )
                        for k in range(KD):
                            nc.tensor.matmul(
                                zps[:, 0:CH],
                                lhsT=w_sb[:, k, ke * P : (ke + 1) * P],
                                rhs=agg[:, k, c * CH : (c + 1) * CH],
                                start=(k == 0),
                                stop=(l == 0 and k == KD - 1),
                            )
                        if l > 0:
                            # += bl[e] via rank-1 matmul (K=1, rhs=ones)
                            nc.tensor.matmul(
                                zps[:, 0:CH],
                                lhsT=bl_sb[0:1, ke * P : (ke + 1) * P],
                                rhs=ones_n[0:1, 0:CH],
                                start=False,
                                stop=True,
                            )
                        dst = (
                            x[:, it, ke, COL0 + c * CH : COL0 + (c + 1) * CH]
                            if l == 0
                            else z_sb[:, ke, c * CH : (c + 1) * CH]
                        )
                        other = (
                            pb_sb[:, ke, c * CH : (c + 1) * CH]
                            if l == 0
                            else x[:, it, ke, COL0 + c * CH : COL0 + (c + 1) * CH]
                        )
                        nc.vector.tensor_tensor(
                            dst, zps[:, 0:CH], other, op=OP.add
                        )
                    if ke % 2 == 1:
                        point()

                if l > 0:
                    pending.append(make_stages(it, z_sb, ga_sb, be_sb))

        # output head: coords.T = Wo.T @ x -> [2, 600] per item, + bo.
        # Head matmuls double as PE filler while the stage pipeline drains.
        def head_chunk(it, c, ob):
            cps = pz.tile([P, 512], dt.float32, tag="zps", name=f"cps_{it}_{c}")
            for k in range(KD):
                nc.tensor.matmul(
                    cps[0:2, 0:CH],
                    lhsT=wo_sb[:, k, :],
                    rhs=x[:, it, k, COL0 + c * CH : COL0 + (c + 1) * CH],
                    start=(k == 0),
                    stop=(k == KD - 1),
                )
            nc.scalar.activation(
                ob[:, c * CH : (c + 1) * CH],
                cps[0:2, 0:CH],
                F.Identity,
                bias=bo_sb[:, 0:1],
            )
            if c == NCH - 1:
                nc.sync.dma_start(outT[it], ob[:])

        head_work = deque()
        for it in range(ITEMS):
            ob = obp.tile([2, N], dt.float32, tag="ob", name=f"ob_{it}")
            for c in range(NCH):
                head_work.append((it, c, ob))

        while pending or head_work:
            # head(it) needs x[it] final: its T7 ran two slots earlier, so
            # drain items 0..5 are ready immediately; 6/7 gate on their stages
            if head_work:
                it, c, ob = head_work.popleft()
                head_chunk(it, c, ob)
            point()

    nc.compile()
    return nc


def _get_nc():
    if "nc" not in _CACHE:
        _CACHE["nc"] = _build_nc()
    return _CACHE["nc"]


def _prep_inputs(features, positions, Wp, bp, pos_tab, Wl, bl, gamma, beta, Wo, bo):
    """Host-side packing: transpose/cast to the device layouts."""
    features = np.ascontiguousarray(np.asarray(features, np.float32))
    positions = np.asarray(positions)
    Wp = np.asarray(Wp, np.float32)
    bp = np.asarray(bp, np.float32)
    pos_tab = np.asarray(pos_tab, np.float32)
    Wl = np.asarray(Wl, np.float32)
    bl = np.asarray(bl, np.float32)
    gamma = np.asarray(gamma, np.float32)
    beta = np.asarray(beta, np.float32)
    Wo = np.asarray(Wo, np.float32)
    bo = np.asarray(bo, np.float32)

    featT = (
        features.transpose(0, 2, 1).reshape(B, KD, P, N).astype(BF16)
    )  # [B, k, p, n]
    # bp + pos_tab[positions]: [B, n, e] -> transposed/bf16 per item
    pe = pos_tab[positions] + bp[None, None, :]
    posbT = pe.transpose(0, 2, 1).reshape(B, KE, P, N).astype(BF16)

    wts = np.concatenate([Wp[None], Wl], axis=0)  # [L+1, d, e]
    wts = wts.reshape(L + 1, KD, P, E).astype(BF16)
    blv = bl.reshape(L, 1, E).astype(BF16)
    gam = np.ascontiguousarray(gamma.reshape(L, KE, P).transpose(0, 2, 1))  # [L, P, KE]
    bet = np.ascontiguousarray(beta.reshape(L, KE, P).transpose(0, 2, 1))
    woT = Wo.reshape(KD, P, 2).astype(BF16)
    bov = bo.reshape(2, 1)

    in_maps = []
    for c in range(NCORES):
        sl = slice(c * ITEMS, (c + 1) * ITEMS)
        in_maps.append(
            {
                "featT": np.ascontiguousarray(featT[sl]),
                "posb": np.ascontiguousarray(posbT[sl]),
                "wts": wts,
                "blv": blv,
                "gam": gam,
                "bet": bet,
                "wo": woT,
                "bo": bov,
            }
        )
    return in_maps


def run_device(in_maps, trace=False, **kwargs):
    """Compile (cached) and run the SPMD kernel; returns BassKernelResults."""
    from concourse import bass_utils

    nc = _get_nc()
    res = bass_utils.run_bass_kernel_spmd(
        nc, in_maps, core_ids=list(range(NCORES)), trace=trace, **kwargs
    )
    return res


def kernel(**inputs) -> np.ndarray:
    in_maps = _prep_inputs(
        inputs["features"],
        inputs["positions"],
        inputs["Wp"],
        inputs["bp"],
        inputs["pos_tab"],
        inputs["Wl"],
        inputs["bl"],
        inputs["gamma"],
        inputs["beta"],
        inputs["Wo"],
        inputs["bo"],
    )
    res = run_device(in_maps, trace=False)
    out = np.empty((B, 600, 2), np.float32)
    for c in range(NCORES):
        o = res.results[c]["outT"]  # [ITEMS, 2, N]
        out[c * ITEMS : (c + 1) * ITEMS] = o.transpose(0, 2, 1)
    out[:, 0, :] = [0.0, 0.0]
    out[:, -1, :] = [600.0, 0.0]
    return out
